# revision 3
# baseline (speedup 1.0000x reference)
"""CKSAAP embedding-average histogram kernel for Trainium2 (8 NeuronCores).

Problem: seq [B=32, L=4096] int codes in [0,20); emb [B, L, D=64] f32; k=7.
out[b, t, a, c, :] = (1/(L-t-1)) * sum_j 0.5*(emb[b,j]+emb[b,j+t+1])
                     over j with seq[b,j]==a and seq[b,j+t+1]==c.

Sharding: data-parallel over batch, 4 batches per core.

Device algorithm (per core), pure one-hot matmul formulation:
  For each (local batch q, gap t): V = emb[j] + emb[j+s]  (s=t+1, f32, DVE),
  split V into bf16 hi+lo halves (exact to ~2^-17), build the pair-code
  one-hot P [positions, 401] on DVE (bf16 is_equal against a remapped iota
  row; code 400 is a junk bin absorbing the invalid tail), then TensorE
  accumulates psum[128, 401] = [V_hi; V_lo]^T @ P over 32 position-chunks
  of 128 (chunk c = positions {32p + c}, so emb loads stay natural/contiguous
  with an 8-column overlap for the shifts).  Eviction sums hi+lo rows and
  scales by 0.5/(L-s) on ScalarE.  Output is written transposed [64, 400];
  the host restores [400, 64] ordering when unsharding.

Codes are remapped by g(n) = n (n<=255) else 256 + 4*(n-256) so every code
is exactly representable in bf16 (no is_equal collisions).
"""

import numpy as np
import ml_dtypes

NUM_AA = 20
L = 4096
D = 64
T = 8           # gaps t = 0..7 (k+1)
B = 32
NCORES = 8
BPC = B // NCORES   # batches per core
NBINS = NUM_AA * NUM_AA  # 400
NCH = 32        # contraction chunks of 128 positions
OVC = 40        # overlap columns per partition (32 + max shift 8)
NCOLS = NBINS + 1  # 401, junk bin last
IOTA_W = 416    # padded iota row width

_CACHE = {}


def _g(n):
    """Injective map of codes 0..400 onto exactly-bf16-representable floats."""
    n = np.asarray(n, dtype=np.int64)
    return np.where(n <= 255, n, 256 + 4 * (n - 256)).astype(np.float32)


def _build(nbt=BPC * T):
    """Build the per-core Bass program. nbt = number of (batch, t) combos."""
    key = ("nc", nbt)
    if key in _CACHE:
        return _CACHE[key]
    import concourse.bass as bass
    import concourse.bacc as bacc
    import concourse.mybir as mybir
    from concourse import tile

    fp32 = mybir.dt.float32
    bf16 = mybir.dt.bfloat16
    AOP = mybir.AluOpType
    ACT = mybir.ActivationFunctionType

    nc = bacc.Bacc()
    emb_ov = nc.dram_tensor("emb_ov", [BPC, 128, OVC * D], fp32, kind="ExternalInput")
    keys = nc.dram_tensor("keys", [nbt, 128, NCH], bf16, kind="ExternalInput")
    iota = nc.dram_tensor("iota", [128, IOTA_W], bf16, kind="ExternalInput")
    outp = nc.dram_tensor("outp", [nbt, D, NBINS], fp32, kind="ExternalOutput")

    with tile.TileContext(nc) as tc:
        with (
            tc.tile_pool(name="const", bufs=1) as cpool,
            tc.tile_pool(name="work", bufs=2) as wpool,
            tc.tile_pool(name="outs", bufs=3) as opool,
            tc.tile_pool(name="psum", bufs=4, space="PSUM") as ppool,
        ):
            emb_sb = cpool.tile([128, BPC * OVC * D], fp32, tag="emb")
            keys_sb = cpool.tile([128, nbt, NCH], bf16, tag="keys")
            iota_sb = cpool.tile([128, IOTA_W], bf16, tag="iota")

            for q in range(BPC):
                nc.sync.dma_start(
                    emb_sb[:, q * OVC * D:(q + 1) * OVC * D], emb_ov[q]
                )
            nc.sync.dma_start(keys_sb[:], keys[:].rearrange("bt p c -> p bt c"))
            nc.sync.dma_start(iota_sb[:], iota[:])

            for bt in range(nbt):
                q, t = divmod(bt, T)
                s = t + 1
                n_t = L - s
                scale = float(0.5 / n_t)
                base = q * OVC * D

                vf = wpool.tile([128, NCH * D], fp32, tag="vf")
                vhl = wpool.tile([128, NCH, 128], bf16, tag="vhl")
                pall = wpool.tile([128, NCH, NCOLS], bf16, tag="pall")

                # V = e_j + e_{j+s}; position j = 32p + c lives at [p, c*64:...]
                nc.vector.tensor_tensor(
                    vf[:],
                    emb_sb[:, base:base + NCH * D],
                    emb_sb[:, base + s * D:base + s * D + NCH * D],
                    AOP.add,
                )
                vf3 = vf[:].rearrange("p (c d) -> p c d", c=NCH)
                # hi = bf16(V) on ScalarE
                nc.scalar.activation(vhl[:, :, 0:D], vf3, ACT.Copy)
                # lo = bf16(V - hi) on VectorE
                nc.vector.tensor_tensor(
                    vhl[:, :, D:2 * D], vf3, vhl[:, :, 0:D], AOP.subtract
                )
                # P[p, c, n] = (iota[n] == keys[p, bt*NCH + c]) in bf16
                nc.vector.tensor_tensor(
                    pall[:],
                    iota_sb[:, 0:NCOLS].rearrange(
                        "p (one n) -> p one n", one=1
                    ).broadcast_to([128, NCH, NCOLS]),
                    keys_sb[:, bt, :].rearrange(
                        "p (c one) -> p c one", one=1
                    ).broadcast_to([128, NCH, NCOLS]),
                    AOP.is_equal,
                )

                ps = ppool.tile([128, NCOLS], fp32, tag="ps")
                for c in range(NCH):
                    nc.tensor.matmul(
                        ps[:],
                        vhl[:, c, :],
                        pall[:, c, :],
                        start=(c == 0),
                        stop=(c == NCH - 1),
                    )

                # DVE cannot mix partition bases across operands; ScalarE can
                # read an offset partition base, so evict hi/lo separately
                # (scale folded into each) and add on DVE.
                his = opool.tile([D, NBINS], fp32, tag="his")
                los = opool.tile([D, NBINS], fp32, tag="los")
                oscaled = opool.tile([D, NBINS], fp32, tag="oscaled")
                nc.scalar.activation(his[:], ps[0:D, 0:NBINS], ACT.Copy, scale=scale)
                nc.scalar.activation(los[:], ps[D:128, 0:NBINS], ACT.Copy, scale=scale)
                nc.vector.tensor_tensor(oscaled[:], his[:], los[:], AOP.add)
                nc.sync.dma_start(outp[bt], oscaled[:])

    nc.finalize()
    _CACHE[key] = nc
    return nc


def _host_prep(seq_np, emb_np, core):
    """Build the per-core input arrays for core index `core`."""
    q0 = core * BPC
    emb_c = emb_np[q0:q0 + BPC]  # [BPC, L, D] f32
    # overlap layout: emb_ov[q, p, :] = emb[q, 32p : 32p+40, :] (zero-padded)
    embp = np.zeros((BPC, L + OVC - 32, D), np.float32)
    embp[:, :L] = emb_c
    idx = (np.arange(128)[:, None] * 32 + np.arange(OVC)[None, :])  # [128, 40]
    emb_ov = embp[:, idx, :].reshape(BPC, 128, OVC * D)

    # keys[bt, p, c] = g(code) for position j = 32p + c
    seq_c = seq_np[q0:q0 + BPC].astype(np.int64)  # [BPC, L]
    keys = np.empty((BPC * T, 128, NCH), np.float32)
    jpos = (np.arange(128)[:, None] * 32 + np.arange(NCH)[None, :])  # [128, 32]
    for q in range(BPC):
        for t in range(T):
            s = t + 1
            n_t = L - s
            a = seq_c[q]
            code = np.full(L, NBINS, np.int64)  # junk bin 400
            code[:n_t] = a[:n_t] * NUM_AA + a[s:s + n_t]
            keys[q * T + t] = _g(code)[jpos]
    keys = keys.astype(ml_dtypes.bfloat16)

    iota_row = np.full(IOTA_W, 9999.0, np.float32)
    iota_row[:NCOLS] = _g(np.arange(NCOLS))
    iota = np.broadcast_to(iota_row, (128, IOTA_W)).astype(ml_dtypes.bfloat16)

    return {
        "emb_ov": np.ascontiguousarray(emb_ov),
        "keys": np.ascontiguousarray(keys),
        "iota": np.ascontiguousarray(iota),
    }


def kernel(seq, emb, k):
    seq_np = np.asarray(seq)
    emb_np = np.asarray(emb, dtype=np.float32)
    kk = int(np.asarray(k))
    assert kk + 1 == T and seq_np.shape == (B, L) and emb_np.shape == (B, L, D)

    from concourse.bass_utils import run_bass_kernel_spmd

    nc = _build()
    in_maps = [_host_prep(seq_np, emb_np, c) for c in range(NCORES)]
    import os
    trace = bool(int(os.environ.get("CK_TRACE", "0")))
    res = run_bass_kernel_spmd(nc, in_maps, list(range(NCORES)), trace=trace)
    global _LAST_EXEC_NS, _LAST_RES
    _LAST_EXEC_NS = res.exec_time_ns
    _LAST_RES = res

    out = np.empty((B, T, NUM_AA, NUM_AA, D), np.float32)
    for c in range(NCORES):
        o = res.results[c]["outp"]  # [nbt, D, NBINS]
        o = np.ascontiguousarray(o.transpose(0, 2, 1))  # [nbt, NBINS, D]
        out[c * BPC:(c + 1) * BPC] = o.reshape(BPC, T, NUM_AA, NUM_AA, D)
    return out


# revision 5
# speedup vs baseline: 1.2756x; 1.2756x over previous
"""CKSAAP embedding-average histogram kernel for Trainium2 (8 NeuronCores).

Problem: seq [B=32, L=4096] int codes in [0,20); emb [B, L, D=64] f32; k=7.
out[b, t, a, c, :] = (1/(L-t-1)) * sum_j 0.5*(emb[b,j]+emb[b,j+t+1])
                     over j with seq[b,j]==a and seq[b,j+t+1]==c.

Sharding: data-parallel over batch, 4 batches per core.

Device algorithm (per core), pure one-hot matmul formulation:
  For each (local batch q, gap t): V = emb[j] + emb[j+s]  (s=t+1, f32, DVE),
  split V into bf16 hi+lo halves (exact to ~2^-17), build the pair-code
  one-hot P [positions, 401] on DVE (bf16 is_equal against a remapped iota
  row; code 400 is a junk bin absorbing the invalid tail), then TensorE
  accumulates psum[128, 401] = [V_hi; V_lo]^T @ P over 32 position-chunks
  of 128 (chunk c = positions {32p + c}, so emb loads stay natural/contiguous
  with an 8-column overlap for the shifts).  Eviction sums hi+lo rows and
  scales by 0.5/(L-s) on ScalarE.  Output is written transposed [64, 400];
  the host restores [400, 64] ordering when unsharding.

Codes are remapped by g(n) = n (n<=255) else 256 + 4*(n-256) so every code
is exactly representable in bf16 (no is_equal collisions).
"""

import numpy as np
import ml_dtypes

NUM_AA = 20
L = 4096
D = 64
T = 8           # gaps t = 0..7 (k+1)
B = 32
NCORES = 8
BPC = B // NCORES   # batches per core
NBINS = NUM_AA * NUM_AA  # 400
NCH = 32        # contraction chunks of 128 positions
OVC = 40        # overlap columns per partition (32 + max shift 8)
NCOLS = NBINS + 1  # 401, junk bin last
IOTA_W = 416    # padded iota row width

_CACHE = {}


def _g(n):
    """Injective map of codes 0..400 onto exactly-bf16-representable floats."""
    n = np.asarray(n, dtype=np.int64)
    return np.where(n <= 255, n, 256 + 4 * (n - 256)).astype(np.float32)


def _build(nbt=BPC * T):
    """Build the per-core Bass program. nbt = number of (batch, t) combos."""
    key = ("nc", nbt)
    if key in _CACHE:
        return _CACHE[key]
    import concourse.bass as bass
    import concourse.bacc as bacc
    import concourse.mybir as mybir
    from concourse import tile

    fp32 = mybir.dt.float32
    bf16 = mybir.dt.bfloat16
    AOP = mybir.AluOpType
    ACT = mybir.ActivationFunctionType

    nc = bacc.Bacc()
    emb_ov = nc.dram_tensor("emb_ov", [BPC, 128, OVC * D], fp32, kind="ExternalInput")
    keys = nc.dram_tensor("keys", [nbt, 128, NCH], fp32, kind="ExternalInput")
    iota = nc.dram_tensor("iota", [128, IOTA_W], bf16, kind="ExternalInput")
    outp = nc.dram_tensor("outp", [nbt, D, NBINS], fp32, kind="ExternalOutput")

    with tile.TileContext(nc) as tc:
        with (
            tc.tile_pool(name="const", bufs=1) as cpool,
            tc.tile_pool(name="work", bufs=2) as wpool,
            tc.tile_pool(name="outs", bufs=3) as opool,
            tc.tile_pool(name="psum", bufs=4, space="PSUM") as ppool,
        ):
            emb_sb = cpool.tile([128, BPC * OVC * D], fp32, tag="emb")
            keys_sb = cpool.tile([128, nbt, NCH], fp32, tag="keys")
            iota_sb = cpool.tile([128, IOTA_W], bf16, tag="iota")

            for q in range(BPC):
                nc.sync.dma_start(
                    emb_sb[:, q * OVC * D:(q + 1) * OVC * D], emb_ov[q]
                )
            nc.sync.dma_start(keys_sb[:], keys[:].rearrange("bt p c -> p bt c"))
            nc.sync.dma_start(iota_sb[:], iota[:])

            for bt in range(nbt):
                q, t = divmod(bt, T)
                s = t + 1
                n_t = L - s
                scale = float(0.5 / n_t)
                base = q * OVC * D

                vf = wpool.tile([128, NCH * D], fp32, tag="vf")
                vhl = wpool.tile([128, NCH, 128], bf16, tag="vhl")
                pall = wpool.tile([128, NCH, IOTA_W], bf16, tag="pall")

                # V = e_j + e_{j+s}; position j = 32p + c lives at [p, c*64:...]
                nc.vector.tensor_tensor(
                    vf[:],
                    emb_sb[:, base:base + NCH * D],
                    emb_sb[:, base + s * D:base + s * D + NCH * D],
                    AOP.add,
                )
                vf3 = vf[:].rearrange("p (c d) -> p c d", c=NCH)
                # hi = bf16(V) on ScalarE
                nc.scalar.activation(vhl[:, :, 0:D], vf3, ACT.Copy)
                # lo = bf16(V - hi) on GpSimd (keeps DVE free for P-gen)
                nc.gpsimd.tensor_tensor(
                    vhl[:, :, D:2 * D], vf3, vhl[:, :, 0:D], AOP.subtract
                )
                # P[p, c, n] = (iota[n] == keys[p, bt*NCH + c]) in bf16.
                # One tensor_scalar per chunk: single-src + bf16 + unit stride
                # hits the DVE 4x perf mode (a broadcast tensor_tensor runs 1x).
                for c in range(NCH):
                    nc.vector.tensor_scalar(
                        pall[:, c, :],
                        iota_sb[:],
                        keys_sb[:, bt, c:c + 1],
                        None,
                        AOP.is_equal,
                    )

                ps = ppool.tile([128, NCOLS], fp32, tag="ps")
                for c in range(NCH):
                    nc.tensor.matmul(
                        ps[:],
                        vhl[:, c, :],
                        pall[:, c, 0:NCOLS],
                        start=(c == 0),
                        stop=(c == NCH - 1),
                    )

                # DVE cannot mix partition bases across operands; ScalarE can
                # read an offset partition base, so evict hi/lo separately
                # (scale folded into each) and add on DVE.
                his = opool.tile([D, NBINS], fp32, tag="his")
                los = opool.tile([D, NBINS], fp32, tag="los")
                oscaled = opool.tile([D, NBINS], fp32, tag="oscaled")
                nc.scalar.activation(his[:], ps[0:D, 0:NBINS], ACT.Copy, scale=scale)
                nc.scalar.activation(los[:], ps[D:128, 0:NBINS], ACT.Copy, scale=scale)
                nc.vector.tensor_tensor(oscaled[:], his[:], los[:], AOP.add)
                nc.sync.dma_start(outp[bt], oscaled[:])

    nc.finalize()
    _CACHE[key] = nc
    return nc


def _host_prep(seq_np, emb_np, core):
    """Build the per-core input arrays for core index `core`."""
    q0 = core * BPC
    emb_c = emb_np[q0:q0 + BPC]  # [BPC, L, D] f32
    # overlap layout: emb_ov[q, p, :] = emb[q, 32p : 32p+40, :] (zero-padded)
    embp = np.zeros((BPC, L + OVC - 32, D), np.float32)
    embp[:, :L] = emb_c
    idx = (np.arange(128)[:, None] * 32 + np.arange(OVC)[None, :])  # [128, 40]
    emb_ov = embp[:, idx, :].reshape(BPC, 128, OVC * D)

    # keys[bt, p, c] = g(code) for position j = 32p + c
    seq_c = seq_np[q0:q0 + BPC].astype(np.int64)  # [BPC, L]
    keys = np.empty((BPC * T, 128, NCH), np.float32)
    jpos = (np.arange(128)[:, None] * 32 + np.arange(NCH)[None, :])  # [128, 32]
    for q in range(BPC):
        for t in range(T):
            s = t + 1
            n_t = L - s
            a = seq_c[q]
            code = np.full(L, NBINS, np.int64)  # junk bin 400
            code[:n_t] = a[:n_t] * NUM_AA + a[s:s + n_t]
            keys[q * T + t] = _g(code)[jpos]
    keys = keys.astype(np.float32)

    iota_row = np.full(IOTA_W, 9999.0, np.float32)
    iota_row[:NCOLS] = _g(np.arange(NCOLS))
    iota = np.broadcast_to(iota_row, (128, IOTA_W)).astype(ml_dtypes.bfloat16)

    return {
        "emb_ov": np.ascontiguousarray(emb_ov),
        "keys": np.ascontiguousarray(keys),
        "iota": np.ascontiguousarray(iota),
    }


def kernel(seq, emb, k):
    seq_np = np.asarray(seq)
    emb_np = np.asarray(emb, dtype=np.float32)
    kk = int(np.asarray(k))
    assert kk + 1 == T and seq_np.shape == (B, L) and emb_np.shape == (B, L, D)

    from concourse.bass_utils import run_bass_kernel_spmd

    nc = _build()
    in_maps = [_host_prep(seq_np, emb_np, c) for c in range(NCORES)]
    import os
    trace = bool(int(os.environ.get("CK_TRACE", "0")))
    res = run_bass_kernel_spmd(nc, in_maps, list(range(NCORES)), trace=trace)
    global _LAST_EXEC_NS, _LAST_RES
    _LAST_EXEC_NS = res.exec_time_ns
    _LAST_RES = res

    out = np.empty((B, T, NUM_AA, NUM_AA, D), np.float32)
    for c in range(NCORES):
        o = res.results[c]["outp"]  # [nbt, D, NBINS]
        o = np.ascontiguousarray(o.transpose(0, 2, 1))  # [nbt, NBINS, D]
        out[c * BPC:(c + 1) * BPC] = o.reshape(BPC, T, NUM_AA, NUM_AA, D)
    return out
